# revision 1
# baseline (speedup 1.0000x reference)
"""DenseCorr2d full kernel for 8 Trainium2 NeuronCores.

Reference computation (per example b):
  corr[(cm*16+ct), y, x] = sum_{dy,dx} tm_edgepad[cm, y+dy, x+dx] * tp[ct, dy, dx]
  out[co, y, x] = bias[co] + sum_{ci,ky,kx} W[co, ci, ky, kx] * corr_zpad[ci, y+ky-1, x+kx-1]

Sharding: data-parallel over batch; core i computes example i entirely.

Stage A runs the dense correlation as 16 PSUM-accumulating matmuls per
spatial tile: the moving operand holds 8 cm-channels x 16 baked x-shifts of
the edge-padded image on the 128 partitions, the stationary is a
block-diagonal arrangement of the template row tp[:, dy, :]; accumulation
over dy happens in PSUM.  corr is kept resident in SBUF zero-padded to
130x130 per 128-channel chunk.

Stage B runs the 3x3 'same' merge conv as 18 PSUM-accumulating matmuls
(9 taps x 2 input-channel chunks) with the tap shift expressed as a free-dim
offset into the padded corr, bias fused into the ScalarE PSUM->SBUF copy.

Matmuls use float32r (full PE rate at N>=256, near-fp32 precision).
"""

from contextlib import ExitStack

import numpy as np

import bass_rust
import concourse.bass as bass
import concourse.tile as tile
from concourse import bacc, mybir
from concourse.bass_utils import run_bass_kernel_spmd

F32 = mybir.dt.float32
F32R = mybir.dt.float32r

N_CORES = 8
# Problem shapes (hardcoded per contract).
B, CT, HT, WT = 8, 16, 16, 16
CM, HM, WM = 16, 128, 128
COUT, K = 64, 3
HP = HM + HT - 1  # 143 padded image rows/cols
ROWS_BLK = 32  # output rows produced per Mblk load
WIN = ROWS_BLK + WT - 1  # 47 input rows needed per block
BAND = 3  # output rows per stage-B matmul band

_CACHE: dict = {}


def _r(ap):
    return ap.bitcast(F32R)


def _emit(ctx: ExitStack, tc, nc, tmp, sa, wst, bia, out):
    const = ctx.enter_context(tc.tile_pool(name="const", bufs=1))
    corrp = ctx.enter_context(tc.tile_pool(name="corrp", bufs=1))

    sa_sb = const.tile([128, 16, 128], F32R, name="sa_sb")
    nc.scalar.dma_start(out=sa_sb[:], in_=sa.ap())
    w_sb = const.tile([128, 18, COUT], F32R, name="w_sb")
    nc.scalar.dma_start(out=w_sb[:], in_=wst.ap())
    b_sb = const.tile([COUT, 1], F32, name="b_sb")
    nc.scalar.dma_start(out=b_sb[:], in_=bia.ap())

    # corr, zero-padded: 2 chunks of 130x130 rows/cols, chunk c = channels
    # [c*128, (c+1)*128) on partitions.
    # 2 elements of tail slack: the last band's kx-shifted windows read (and
    # discard) up to 2 elements past the padded grid.
    corr_sb = corrp.tile([128, 2 * 130 * 130 + 2], F32R, name="corr_sb")
    corr_flat = corr_sb[:]
    corr = corr_sb[:, : 2 * 130 * 130].rearrange("p (a b) -> p a b", a=2 * 130)
    # Zero the padding borders by DMA from a host-supplied zeros tensor
    # (memset can't emit float32r, and the fp32r provenance verifier
    # rejects fp32-written bytes feeding fp32r matmuls).
    zz = _CACHE["zz_handle"]
    nc.scalar.dma_start(out=corr_sb[:, 2 * 130 * 130 :], in_=zz.ap()[:, :2])
    for c in range(2):
        nc.scalar.dma_start(out=corr[:, c * 130, :], in_=zz.ap()[:, :130])
        nc.scalar.dma_start(out=corr[:, c * 130 + 129, :], in_=zz.ap()[:, :130])
        nc.scalar.dma_start(
            out=corr[:, c * 130 : (c + 1) * 130, 0], in_=zz.ap()[:, :130]
        )
        nc.scalar.dma_start(
            out=corr[:, c * 130 : (c + 1) * 130, 129], in_=zz.ap()[:, :130]
        )

    # ---- Stage A ----
    # Partition (g, dy) holds image cm=8h+g Y-SHIFTED by dy rows, full-width
    # 143-element rows (so the (y,x) free dims merge and each block load is
    # ONE contiguous-run 128-partition DMA at full fabric width).  The dx
    # shift of the correlation becomes a free-dim offset of the moving
    # operand; accumulation over dx happens in PSUM with a dx-indexed
    # block-diagonal stationary.
    with (
        tc.tile_pool(name="mblk", bufs=2) as mpool,
        tc.tile_pool(name="psA", bufs=8, space="PSUM") as psA,
    ):
        for h in range(2):  # cm halves (8 channels each)
            for blk in range(HM // ROWS_BLK):
                r0 = ROWS_BLK * blk
                mt = mpool.tile([128, ROWS_BLK, HP], F32R, name="mt", tag="mt")
                # src[p=(g,dy), y, x] = tm_pad[8h+g, r0+dy+y, x]
                src = tmp.ap()[8 * h : 8 * h + 8, r0 : r0 + ROWS_BLK, :]
                src.ap = bass_rust.VecI64Pair(
                    [[HP * HP, 8], [HP, 16], [1, ROWS_BLK * HP]]
                )
                nc.sync.dma_start(out=mt[:], in_=src)
                pts = [
                    psA.tile([128, 4, WM], F32, name=f"pA{sp}", tag="pA")
                    for sp in range(8)
                ]
                for dx in range(WT):
                    for sp in range(8):
                        nc.tensor.matmul(
                            pts[sp][:],
                            sa_sb[:, dx, :],
                            mt[:, 4 * sp : 4 * sp + 4, dx : dx + WM],
                            start=(dx == 0),
                            stop=(dx == WT - 1),
                        )
                for sp in range(8):
                    rr = h * 130 + r0 + 4 * sp + 1
                    nc.vector.tensor_copy(
                        corr[:, rr : rr + 4, 1:129], pts[sp][:]
                    )

    # ---- Stage B ----
    # Stage B processes bands in PAIRS: band A accumulates on PSUM
    # partitions 0:64 (tile_position col 0), band B on partitions 64:128
    # (tile_position col 64) of the same bank.  The two matmul streams use
    # disjoint PE column groups and run concurrently (~2x stage-B
    # throughput at M=64).
    with (
        tc.tile_pool(name="psB", bufs=4, space="PSUM") as psB,
        tc.tile_pool(name="outp", bufs=2) as outp,
    ):
        n_bands = (HM + BAND - 1) // BAND
        GRP = 8  # bands per output DMA batch
        band = 0
        ot = None
        ot_base = 0
        for band in range(0, n_bands, 2):
            if band % GRP == 0:
                if ot is not None:
                    g_rows = BAND * band - ot_base
                    nc.scalar.dma_start(
                        out=out.ap()[:, ot_base : ot_base + g_rows, :],
                        in_=ot[:, :g_rows, :],
                    )
                ot = outp.tile([COUT, GRP * BAND, WM], F32, name="ot", tag="ot")
                ot_base = BAND * band
            pair = [b for b in (band, band + 1) if b < n_bands]
            for bi in pair:
                y0 = BAND * bi
                rows = min(BAND, HM - y0)
                n = rows * 130
                pb = psB.tile([COUT, BAND * 130], F32, name="pb", tag="pb")
                for c in range(2):
                    for s in range(9):
                        ky, kx = divmod(s, 3)
                        off = (c * 130 + y0 + ky) * 130 + kx
                        nc.tensor.matmul(
                            pb[:, :n],
                            w_sb[:, c * 9 + s, :],
                            corr_flat[:, off : off + n],
                            start=(c == 0 and s == 0),
                            stop=(c == 1 and s == 8),
                        )
                nc.scalar.activation(
                    ot[:, y0 - ot_base : y0 - ot_base + rows, :],
                    pb[:, : rows * 130].rearrange("p (a b) -> p a b", b=130)[
                        :, :, 0:128
                    ],
                    mybir.ActivationFunctionType.Identity,
                    bias=b_sb[:, 0:1],
                )
        g_rows = HM - ot_base
        nc.scalar.dma_start(
            out=out.ap()[:, ot_base : ot_base + g_rows, :],
            in_=ot[:, :g_rows, :],
        )


def _build(loop_n: int = 1):
    nc = bacc.Bacc("TRN2", target_bir_lowering=False, debug=False)
    tmp = nc.dram_tensor("tmp", [CM, HP, HP], F32R, kind="ExternalInput")
    sa = nc.dram_tensor("sa", [128, 16, 128], F32R, kind="ExternalInput")
    wst = nc.dram_tensor("wst", [128, 18, COUT], F32R, kind="ExternalInput")
    bia = nc.dram_tensor("bias", [COUT, 1], F32, kind="ExternalInput")
    _CACHE["zz_handle"] = nc.dram_tensor(
        "zz", [128, 130], F32R, kind="ExternalInput"
    )
    out = nc.dram_tensor("out", [COUT, HM, WM], F32, kind="ExternalOutput")
    with tile.TileContext(nc) as tc, ExitStack() as ctx:
        if loop_n > 1:
            with tc.For_i(0, loop_n, 1):
                _emit(ctx, tc, nc, tmp, sa, wst, bia, out)
        else:
            _emit(ctx, tc, nc, tmp, sa, wst, bia, out)
    nc.compile()
    return nc


def _get_nc():
    if "nc" not in _CACHE:
        _CACHE["nc"] = _build()
    return _CACHE["nc"]


def _host_prep(template, tomatch, W, b):
    template = np.ascontiguousarray(template, dtype=np.float32)
    tomatch = np.ascontiguousarray(tomatch, dtype=np.float32)
    W = np.ascontiguousarray(W, dtype=np.float32)
    b = np.ascontiguousarray(b, dtype=np.float32)

    tm_pad = np.pad(
        tomatch, ((0, 0), (0, 0), (0, HT - 1), (0, WT - 1)), mode="edge"
    )  # [B, CM, 143, 143]

    # sa[b, g*16+dy, dx, g*16+ct] = template[b, ct, dy, dx]
    sa = np.zeros((B, 128, 16, 128), np.float32)
    tpT = template.transpose(0, 2, 3, 1)  # [b, dy, dx, ct]
    for g in range(8):
        sa[:, g * 16 : g * 16 + 16, :, g * 16 : g * 16 + 16] = tpT

    # wst[k, c*9 + ky*3 + kx, co] = W[co, c*128+k, ky, kx]
    wst = np.ascontiguousarray(
        W.reshape(COUT, 2, 128, K, K).transpose(2, 1, 3, 4, 0).reshape(128, 18, COUT)
    )
    bias = np.ascontiguousarray(b.reshape(COUT, 1))
    zz = np.zeros((128, 130), np.float32)
    return tm_pad, sa, wst, bias, zz


def kernel(template, tomatch, W, b):
    tm_pad, sa, wst, bias, zz = _host_prep(template, tomatch, W, b)
    nc = _get_nc()
    in_maps = [
        {"tmp": tm_pad[i], "sa": sa[i], "wst": wst, "bias": bias, "zz": zz}
        for i in range(N_CORES)
    ]
    res = run_bass_kernel_spmd(nc, in_maps, list(range(N_CORES)))
    return np.stack([res.results[i]["out"] for i in range(N_CORES)])

